# revision 2
# baseline (speedup 1.0000x reference)
"""Multi-head attention (B=128, T=256, D=512, H=8, HD=64) on 8 TRN2 NeuronCores.

Strategy: data-parallel over batch (16 batches per core), full weights
replicated.  Per-core Bass/Tile kernel works in "transposed" space:

  xT[d, t]      <- PE-transpose of x[t, d]                (per batch)
  QT/KT[hd, t]  <- Wq/Wk-pair.T @ xT (f32r matmuls, head pairs packed M=128)
  V[s, hd4]     <- xT-chunk.T @ Wv (4 heads per matmul, natural layout)
  scT[s, t]     <- KT-slice.T @ QT  == scores^T            (per head)
  expT          <- exp(0.125 * scT) (ACT), causal-masked via gpsimd
                   affine_select (keep t >= s, else 0)
  colsum_bc     <- ones[128,64].T @ expT  (PE broadcast of softmax denom)
  oT[hd, t]     <- V-slice.T @ expT ; concatT = oT * recip(colsum_bc)
  out[t, :]     <- concatT-chunk.T @ Wo + bo (bias pre-broadcast via PE)

All matmul operands are float32r (TF32-ish PE fast path, 1 cyc/row at
N>=256); every f32r operand is produced by a compute op (DVE/ACT copy or
activation) to satisfy the BIR verifier's rounding rule.
"""
import numpy as np
from contextlib import ExitStack

import jax
import concourse.bass as bass
import concourse.mybir as mybir
import concourse.tile as tile
from concourse import bacc
from concourse.masks import make_identity

F32 = mybir.dt.float32
F32R = mybir.dt.float32r
EXP = mybir.ActivationFunctionType.Exp

NCORES = 8
B, T, D, H, HD = 128, 256, 512, 8, 64
BL = B // NCORES          # batches per core
NCH = D // 128            # 4 contraction chunks of 128
NPAIR = H // 2            # 4 head pairs
SCALE = float(HD) ** -0.5  # 0.125


def _emit(nc):
    x_d = nc.dram_tensor("x", [BL, T, D], F32, kind="ExternalInput")
    wq_d = nc.dram_tensor("Wq", [H, D, HD], F32, kind="ExternalInput")
    wk_d = nc.dram_tensor("Wk", [H, D, HD], F32, kind="ExternalInput")
    wv_d = nc.dram_tensor("Wv", [H, D, HD], F32, kind="ExternalInput")
    wo_d = nc.dram_tensor("Wo", [D, D], F32, kind="ExternalInput")
    bo_d = nc.dram_tensor("bo", [1, D], F32, kind="ExternalInput")
    out_d = nc.dram_tensor("out", [BL, T, D], F32, kind="ExternalOutput")

    with tile.TileContext(nc) as tc:
        with ExitStack() as ctx:
            const = ctx.enter_context(tc.tile_pool(name="const", bufs=1))
            wst = ctx.enter_context(tc.tile_pool(name="wst", bufs=2))
            xp = ctx.enter_context(tc.tile_pool(name="xp", bufs=4))
            xtp = ctx.enter_context(tc.tile_pool(name="xtp", bufs=2))
            qkvp = ctx.enter_context(tc.tile_pool(name="qkvp", bufs=2))
            expp = ctx.enter_context(tc.tile_pool(name="expp", bufs=3))
            recp = ctx.enter_context(tc.tile_pool(name="recp", bufs=3))
            osbp = ctx.enter_context(tc.tile_pool(name="osbp", bufs=3))
            # PSUM: 8 banks total -> big(2) + proj(2) + sc(2) + att(2)
            psum = ctx.enter_context(tc.tile_pool(name="ps", bufs=2, space="PSUM"))

            # ---- constants -------------------------------------------------
            ident = const.tile([128, 128], F32)
            make_identity(nc, ident)
            ones_st = const.tile([128, 64], F32)
            nc.gpsimd.memset(ones_st, 1.0)
            ones_cs = const.tile([128, 64], F32R)
            nc.vector.tensor_copy(ones_cs, ones_st)
            ones1 = const.tile([1, 128], F32)
            nc.gpsimd.memset(ones1, 1.0)
            bo_sb = const.tile([1, D], F32)
            nc.sync.dma_start(bo_sb, bo_d[:, :])
            # bias broadcast to all 128 partitions via K=1 plain-f32 matmul
            bo_ps = psum.tile([128, D], F32, tag="big")
            nc.tensor.matmul(bo_ps, ones1, bo_sb, start=True, stop=True)
            bo_bc = const.tile([128, D], F32)
            nc.vector.tensor_copy(bo_bc, bo_ps)

            # ---- weights: DMA f32 staging -> round-copy to f32r ------------
            # layout [128 (d within chunk), NCH (d chunk), H*HD (h*64+hd)]
            w_r = {}
            for nm, wd in (("q", wq_d), ("k", wk_d), ("v", wv_d)):
                stg = wst.tile([128, NCH, D], F32, tag="wstage", name=f"stg_{nm}")
                for c in range(NCH):
                    for h in range(H):
                        nc.sync.dma_start(
                            stg[:, c, h * HD:(h + 1) * HD],
                            wd[h, c * 128:(c + 1) * 128, :],
                        )
                wr = const.tile([128, NCH, D], F32R, name=f"w_{nm}")
                nc.vector.tensor_copy(wr, stg)
                w_r[nm] = wr
            stg_o = wst.tile([128, NCH, D], F32, tag="wstage")
            for c in range(NCH):
                nc.sync.dma_start(stg_o[:, c, :], wo_d[c * 128:(c + 1) * 128, :])
            wo_r = const.tile([128, NCH, D], F32R)
            nc.vector.tensor_copy(wo_r, stg_o)

            # ---- per-batch state carried across the pipelined head loop ----
            def load_and_project(b):
                """Phases A+B for batch b; returns (xT, QT, KT, V) tiles."""
                xt = []
                for tci in range(2):
                    x_t = xp.tile([128, D], F32, tag="x", name=f"x_{b}_{tci}")
                    nc.sync.dma_start(x_t, x_d[b, tci * 128:(tci + 1) * 128, :])
                    xt.append(x_t)
                xT = xtp.tile([128, NCH, T], F32R, tag="xT", name=f"xT_{b}")
                for tci in range(2):
                    for c in range(NCH):
                        tp_ps = psum.tile([128, 128], F32, tag="big",
                                          name=f"tp_{b}_{tci}_{c}")
                        nc.tensor.transpose(
                            tp_ps, xt[tci][:, c * 128:(c + 1) * 128], ident)
                        nc.vector.tensor_copy(
                            xT[:, c, tci * 128:(tci + 1) * 128], tp_ps)

                QT = qkvp.tile([128, NPAIR, T], F32R, tag="qt", name=f"QT_{b}")
                KT = qkvp.tile([128, NPAIR, T], F32R, tag="kt", name=f"KT_{b}")
                for nm, dst, eng in (("q", QT, nc.scalar), ("k", KT, nc.vector)):
                    for p in range(NPAIR):
                        pj = psum.tile([128, T], F32, tag="proj",
                                       name=f"pj_{nm}_{b}_{p}")
                        for c in range(NCH):
                            nc.tensor.matmul(
                                pj,
                                w_r[nm][:, c, p * 128:(p + 1) * 128],
                                xT[:, c, :],
                                start=(c == 0), stop=(c == NCH - 1),
                            )
                        if eng is nc.scalar:
                            nc.scalar.copy(dst[:, p, :], pj)
                        else:
                            nc.vector.tensor_copy(dst[:, p, :], pj)

                # V natural [s, h*64+hd]: free layout (sc, 4-head group)
                V = qkvp.tile([128, 2, D], F32R, tag="v", name=f"V_{b}")
                for sc in range(2):
                    for q in range(2):
                        pj = psum.tile([128, T], F32, tag="proj",
                                       name=f"pj_v_{b}_{sc}_{q}")
                        for c in range(NCH):
                            nc.tensor.matmul(
                                pj,
                                xT[:, c, sc * 128:(sc + 1) * 128],
                                w_r["v"][:, c, q * 256:(q + 1) * 256],
                                start=(c == 0), stop=(c == NCH - 1),
                            )
                        nc.vector.tensor_copy(V[:, sc, q * 256:(q + 1) * 256], pj)
                return QT, KT, V

            def emit_scores(b, h, QT, KT):
                """scT psum + exp + mask -> returns masked expT tile."""
                p, hh = divmod(h, 2)
                pb = hh * HD
                sc_ps = psum.tile([128, 2, T], F32, tag="sc",
                                  name=f"sc_{b}_{h}")
                for sc in range(2):
                    nc.tensor.matmul(
                        sc_ps[:, sc, :],
                        KT[pb:pb + HD, p, sc * 128:(sc + 1) * 128],
                        QT[pb:pb + HD, p, :],
                        start=True, stop=True,
                    )
                expT = expp.tile([128, 2, T], F32R, tag="expT",
                                 name=f"expT_{b}_{h}")
                for sc in range(2):
                    nc.scalar.activation(expT[:, sc, :], sc_ps[:, sc, :],
                                         EXP, scale=SCALE)
                    # causal: keep where t - s >= 0  (iota = f - p - 128*sc)
                    nc.gpsimd.affine_select(
                        out=expT[:, sc, :], in_=expT[:, sc, :],
                        compare_op=mybir.AluOpType.is_ge,
                        fill=0.0, base=-128 * sc,
                        pattern=[[1, T]], channel_multiplier=-1,
                    )
                return expT

            def emit_tail(b, h, V, expT, catT):
                """colsum -> recip; oT; normalized write into catT."""
                p, hh = divmod(h, 2)
                cs_ps = psum.tile([HD, T], F32, tag="att", name=f"cs_{b}_{h}")
                for sc in range(2):
                    nc.tensor.matmul(cs_ps, ones_cs, expT[:, sc, :],
                                     start=(sc == 0), stop=(sc == 1))
                recip = recp.tile([HD, T], F32, tag="rec", name=f"rec_{b}_{h}")
                nc.vector.reciprocal(recip, cs_ps)
                ot_ps = psum.tile([HD, T], F32, tag="att", name=f"ot_{b}_{h}")
                for sc in range(2):
                    nc.tensor.matmul(ot_ps, V[:, sc, h * HD:(h + 1) * HD],
                                     expT[:, sc, :],
                                     start=(sc == 0), stop=(sc == 1))
                nc.vector.tensor_mul(catT[hh * HD:(hh + 1) * HD, p, :],
                                     ot_ps, recip)

            def emit_outproj(b, catT):
                for tci in range(2):
                    po = psum.tile([128, D], F32, tag="big",
                                   name=f"po_{b}_{tci}")
                    for c in range(NCH):
                        nc.tensor.matmul(
                            po,
                            catT[:, c, tci * 128:(tci + 1) * 128],
                            wo_r[:, c, :],
                            start=(c == 0), stop=(c == NCH - 1),
                        )
                    osb = osbp.tile([128, D], F32, tag="osb",
                                    name=f"osb_{b}_{tci}")
                    nc.vector.tensor_add(osb, po, bo_bc)
                    nc.sync.dma_start(
                        out_d[b, tci * 128:(tci + 1) * 128, :], osb)

            # ---- main loop: heads software-pipelined (scores one ahead) ----
            for b in range(BL):
                QT, KT, V = load_and_project(b)
                catT = qkvp.tile([128, NPAIR, T], F32R, tag="cat",
                                 name=f"catT_{b}")
                prev = None
                for h in range(H + 1):
                    if h < H:
                        expT = emit_scores(b, h, QT, KT)
                    if prev is not None:
                        emit_tail(b, prev[0], V, prev[1], catT)
                    prev = (h, expT) if h < H else None
                emit_outproj(b, catT)

    nc.compile()
    return nc


_CACHE = {}


def _get_runner():
    """Build the bass module once and a cached jitted SPMD executor."""
    if "run" in _CACHE:
        return _CACHE["run"]

    from jax.sharding import Mesh, PartitionSpec
    from jax.experimental.shard_map import shard_map
    from concourse.bass2jax import (
        _bass_exec_p, install_neuronx_cc_hook, partition_id_tensor)
    import concourse.mybir as mybir_

    nc = bacc.Bacc("TRN2", target_bir_lowering=False, debug=False)
    _emit(nc)

    install_neuronx_cc_hook()

    partition_name = (nc.partition_id_tensor.name
                      if nc.partition_id_tensor else None)
    in_names, out_names, out_avals, zero_outs = [], [], [], []
    for alloc in nc.m.functions[0].allocations:
        if not isinstance(alloc, mybir_.MemoryLocationSet):
            continue
        name = alloc.memorylocations[0].name
        if alloc.kind == "ExternalInput":
            if name != partition_name:
                in_names.append(name)
        elif alloc.kind == "ExternalOutput":
            out_names.append(name)
            shape = tuple(alloc.tensor_shape)
            dtype = mybir_.dt.np(alloc.dtype)
            out_avals.append(jax.core.ShapedArray(shape, dtype))
            zero_outs.append(np.zeros((NCORES * shape[0], *shape[1:]), dtype))
    n_params = len(in_names)
    all_names = in_names + out_names
    if partition_name is not None:
        all_names = all_names + [partition_name]

    def _body(*args):
        operands = list(args)
        if partition_name is not None:
            operands.append(partition_id_tensor())
        outs = _bass_exec_p.bind(
            *operands,
            out_avals=tuple(out_avals),
            in_names=tuple(all_names),
            out_names=tuple(out_names),
            lowering_input_output_aliases=(),
            sim_require_finite=True,
            sim_require_nnan=True,
            nc=nc,
        )
        return tuple(outs)

    devices = jax.devices()[:NCORES]
    mesh = Mesh(np.asarray(devices), ("core",))
    n_outs = len(out_names)
    sharded = jax.jit(
        shard_map(
            _body, mesh=mesh,
            in_specs=(PartitionSpec("core"),) * (n_params + n_outs),
            out_specs=(PartitionSpec("core"),) * n_outs,
            check_rep=False,
        ),
        donate_argnums=tuple(range(n_params, n_params + n_outs)),
        keep_unused=True,
    )

    def run(in_map_global):
        """in_map_global: name -> global (NCORES*dim0, ...) np array."""
        args = [in_map_global[n] for n in in_names]
        zeros = [np.zeros_like(z) for z in zero_outs]
        outs = sharded(*args, *zeros)
        return {n: np.asarray(outs[i]) for i, n in enumerate(out_names)}

    _CACHE["run"] = run
    return run


def kernel(x, Wq, Wk, Wv, Wo, bo):
    run = _get_runner()
    rep = lambda w: np.concatenate([np.asarray(w, np.float32)] * NCORES, axis=0)
    in_map = {
        "x": np.ascontiguousarray(np.asarray(x, np.float32)),      # [128,256,512]
        "Wq": rep(Wq), "Wk": rep(Wk), "Wv": rep(Wv),               # [8H,...]
        "Wo": rep(Wo),                                             # [8*512,512]
        "bo": rep(np.asarray(bo, np.float32).reshape(1, D)),       # [8,512]
    }
    out = run(in_map)["out"]                                       # [128,256,512]
    return out.astype(np.float32)
